# revision 21
# baseline (speedup 1.0000x reference)
"""DividedAttentionSublayer on 8 TRN2 NeuronCores.

Sharding: data-parallel over batch (B=8 -> 1 batch element per core),
weights / pos_emb replicated. Per core the attention runs in a
transposed layout (k on partitions, q on free dim) so attn@V needs no
attn-matrix transpose; softmax denominators come from a ones-column
augmented V; the relative-position band uses a skewed-stride DRAM
re-read (rel-shift trick) plus three 128x128 PE transposes per
(head, q-tile); clamped tails (|k-q| > 128) are folded into the logits
matmul via augmented contraction rows (Lo at row 64, Hi-Lo at row 65).
"""
import sys

sys.path.insert(0, "/opt/trn_rl_repo")

import numpy as np
import ml_dtypes
import concourse.bass as bass
import concourse.mybir as mybir
from concourse import bacc
from concourse.tile import TileContext
from concourse.bass import AP
from concourse.bass_utils import run_bass_kernel_spmd

F32 = mybir.dt.float32
F32R = mybir.dt.float32r
BF16 = mybir.dt.bfloat16
EXP = mybir.ActivationFunctionType.Exp

B, L, D = 8, 1024, 1024
H, DH = 16, 64
NT = L // 128
SCALE = float(np.sqrt(D / H))
JW = 257
EPW = 264
PADW = 512
GW = 384

_NC = None


def _build():
    nc = bacc.Bacc(None, target_bir_lowering=False)

    xqt = nc.dram_tensor("xqt", [D, L], F32R, kind="ExternalInput")
    xkt = nc.dram_tensor("xkt", [D, L], F32R, kind="ExternalInput")
    xvt = nc.dram_tensor("xvt", [D, L], F32R, kind="ExternalInput")
    wq = nc.dram_tensor("wq", [D, D], F32R, kind="ExternalInput")
    wk = nc.dram_tensor("wk", [D, D], F32R, kind="ExternalInput")
    wv = nc.dram_tensor("wv", [D, D], F32R, kind="ExternalInput")
    wo = nc.dram_tensor("wo", [D, D], F32R, kind="ExternalInput")
    bqc = nc.dram_tensor("bqc", [128, NT], F32, kind="ExternalInput")
    bkc = nc.dram_tensor("bkc", [128, NT], F32, kind="ExternalInput")
    bvr = nc.dram_tensor("bvr", [1, D], F32R, kind="ExternalInput")
    boc = nc.dram_tensor("boc", [128, NT], F32, kind="ExternalInput")
    ept = nc.dram_tensor("ept", [DH, EPW], BF16, kind="ExternalInput")
    ep2 = nc.dram_tensor("ep2", [DH, 2], BF16, kind="ExternalInput")
    mkb = nc.dram_tensor("mkb", [128, NT], F32, kind="ExternalInput")
    idn = nc.dram_tensor("idn", [128, 128], BF16, kind="ExternalInput")
    vob = nc.dram_tensor("vob", [128, H], BF16, kind="ExternalInput")
    onr = nc.dram_tensor("onr", [1, 128], F32R, kind="ExternalInput")
    outt = nc.dram_tensor("outt", [D, L], F32, kind="ExternalOutput")

    r = lambda t: t.rearrange("(c p) l -> c p l", p=128)
    xqt_c, xkt_c, xvt_c = r(xqt[:]), r(xkt[:]), r(xvt[:])
    wq_c, wk_c, wv_c, wo_c = r(wq[:]), r(wk[:]), r(wv[:]), r(wo[:])

    with TileContext(nc) as tc:
        with (
            tc.tile_pool(name="persist", bufs=1) as pp,
            tc.tile_pool(name="qth", bufs=16) as pool_qth,
            tc.tile_pool(name="kth", bufs=16) as pool_kth,
            tc.tile_pool(name="vsb", bufs=8) as pool_v,
            tc.tile_pool(name="ct", bufs=8) as pool_ct,
        ):
            ept_b = pp.tile([DH, EPW], BF16, tag="eptb")
            ep2_b = pp.tile([DH, 2], BF16, tag="ep2b")
            bq_sb = pp.tile([128, NT], F32, tag="bq")
            bk_sb = pp.tile([128, NT], F32, tag="bk")
            bv_sb = pp.tile([1, D], F32R, tag="bv")
            bo_sb = pp.tile([128, NT], F32, tag="bo")
            mk_sb = pp.tile([128, NT], F32, tag="mk")
            ident = pp.tile([128, 128], BF16, tag="ident")
            ones_row = pp.tile([1, 128], F32R, tag="onr")
            qth = [pool_qth.tile([66, L], BF16, tag="qth", name=f"qth{i}") for i in range(H)]
            kth = [pool_kth.tile([66, L], BF16, tag="kth", name=f"kth{i}") for i in range(H)]
            v_sb = [pool_v.tile([128, H * 65], BF16, tag="v", name=f"vsb{i}") for i in range(NT)]
            ct = [pool_ct.tile([128, L], F32R, tag="ct", name=f"ct{i}") for i in range(NT)]

            for h in range(H):
                nc.vector.memset(kth[h][64:66, :], 1.0)

            def _load_consts():
                for t, src in ((bq_sb, bqc), (bk_sb, bkc), (bv_sb, bvr), (bo_sb, boc),
                               (ept_b, ept), (ep2_b, ep2), (mk_sb, mkb), (ident, idn),
                               (ones_row, onr)):
                    nc.sync.dma_start(t[:], src[:])
                for lt in range(NT):
                    nc.sync.dma_start(
                        v_sb[lt][:].rearrange("p (h c) -> p h c", c=65)[:, :, 64:65],
                        vob[:].rearrange("p (h c) -> p h c", c=1),
                    )

            # =========== Q/K projections (transposed outputs) ===========
            with (
                tc.tile_pool(name="xin", bufs=8) as pool_x,
                tc.tile_pool(name="win", bufs=16) as pool_w,
                tc.tile_pool(name="pps", bufs=4, space="PSUM") as pool_ps,
            ):
                for which in range(2):
                    x_c = (xqt_c, xkt_c)[which]
                    w_c = (wq_c, wk_c)[which]
                    dst = (qth, kth)[which]
                    bcol = (bq_sb, bk_sb)[which]
                    x_sb = [pool_x.tile([128, L], F32R, tag="x", name=f"xsb{c}") for c in range(NT)]
                    for c in range(NT):
                        nc.sync.dma_start(x_sb[c][:], x_c[c])
                    if which == 0:
                        _load_consts()
                    for i in range(NT):
                        w_sb = [pool_w.tile([128, 128], F32R, tag="w", name=f"wsb{c}") for c in range(NT)]
                        for c in range(NT):
                            nc.sync.dma_start(w_sb[c][:], w_c[c][:, 128 * i : 128 * i + 128])
                        for lh in range(2):
                            ps = pool_ps.tile([128, 512], F32, tag="ps")
                            for c in range(NT):
                                nc.tensor.matmul(
                                    ps[:],
                                    w_sb[c][:],
                                    x_sb[c][:, 512 * lh : 512 * lh + 512],
                                    start=(c == 0),
                                    stop=(c == NT - 1),
                                )
                            for half in range(2):
                                h = 2 * i + half
                                nc.vector.tensor_scalar_add(
                                    dst[h][0:64, 512 * lh : 512 * lh + 512],
                                    ps[64 * half : 64 * half + 64, :],
                                    bcol[64 * half : 64 * half + 64, i : i + 1],
                                )

                # ---- V projection (natural layout, bias via K=1 ones matmul) ----
                x_sb = [pool_x.tile([128, L], F32R, tag="x", name=f"xsb{c}") for c in range(NT)]
                for c in range(NT):
                    nc.sync.dma_start(x_sb[c][:], xvt_c[c])
                for dh_ in range(2):
                    w_sb = [pool_w.tile([128, 512], F32R, tag="w", name=f"wvsb{c}") for c in range(NT)]
                    for c in range(NT):
                        nc.sync.dma_start(w_sb[c][:], wv_c[c][:, 512 * dh_ : 512 * dh_ + 512])
                    for lt in range(NT):
                        ps = pool_ps.tile([128, 512], F32, tag="ps")
                        for c in range(NT):
                            nc.tensor.matmul(
                                ps[:],
                                x_sb[c][:, 128 * lt : 128 * lt + 128],
                                w_sb[c][:],
                                start=(c == 0),
                                stop=False,
                            )
                        nc.tensor.matmul(
                            ps[:],
                            ones_row[:],
                            bv_sb[0:1, 512 * dh_ : 512 * dh_ + 512],
                            start=False,
                            stop=True,
                        )
                        nc.vector.tensor_copy(
                            v_sb[lt][:].rearrange("p (h c) -> p h c", c=65)[
                                :, 8 * dh_ : 8 * dh_ + 8, 0:64
                            ],
                            ps[:].rearrange("p (a b) -> p a b", a=8),
                        )

            # =========== attention ===========
            with (
                tc.tile_pool(name="attn", bufs=10) as pool_attn,
                tc.tile_pool(name="scratch", bufs=4) as pool_s,
                tc.tile_pool(name="gts", bufs=10) as pool_gt,
                tc.tile_pool(name="psA", bufs=2, space="PSUM") as pool_psA,
                tc.tile_pool(name="psG", bufs=1, space="PSUM") as pool_psG,
                tc.tile_pool(name="psL", bufs=2, space="PSUM") as pool_psL,
                tc.tile_pool(name="psV", bufs=1, space="PSUM") as pool_psV,
                tc.tile_pool(name="dram", bufs=4, space="DRAM") as pool_d,
            ):
                for h in range(H):
                    q = qth[h]
                    k = kth[h]
                    for lh in range(2):
                        ps2 = pool_psA.tile([2, 512], F32, tag="mm")
                        nc.tensor.matmul(
                            ps2[:], ep2_b[:], q[0:64, 512 * lh : 512 * lh + 512],
                            start=True, stop=True,
                        )
                        nc.vector.tensor_copy(q[64:66, 512 * lh : 512 * lh + 512], ps2[:])

                    gt = []
                    for m in range(NT):
                        psp = pool_psA.tile([128, EPW], F32, tag="mm")
                        nc.tensor.matmul(
                            psp[:], q[0:64, 128 * m : 128 * m + 128], ept_b[:],
                            start=True, stop=True,
                        )
                        pex = pool_s.tile([128, PADW], BF16, tag="pex")
                        nc.scalar.activation(pex[:, 127 : 127 + JW], psp[:, 0:JW], EXP)
                        nc.vector.tensor_copy(
                            pex[:, 0:127], pex[:, 127:128].to_broadcast([128, 127])
                        )
                        nc.vector.tensor_copy(
                            pex[:, 384:512], pex[:, 383:384].to_broadcast([128, 128])
                        )
                        dpad = pool_d.tile([128, PADW], BF16, tag="dpad")
                        nc.sync.dma_start(dpad[:], pex[:])
                        g = pool_s.tile([128, GW], BF16, tag="g")
                        nc.sync.dma_start(
                            g[:], AP(dpad.tensor, dpad.offset + 127, [[PADW - 1, 128], [1, GW]])
                        )
                        gps = pool_psG.tile([128, GW], BF16, tag="gt_ps")
                        j0 = 1 if m == 0 else 0
                        j1 = 2 if m == NT - 1 else 3
                        for j in range(j0, j1):
                            nc.tensor.transpose(
                                gps[:, 128 * j : 128 * j + 128],
                                g[:, 128 * j : 128 * j + 128],
                                ident[:],
                            )
                        gsb = pool_gt.tile([128, GW], BF16, tag="gt")
                        nc.vector.tensor_copy(
                            gsb[:, 128 * j0 : 128 * j1], gps[:, 128 * j0 : 128 * j1]
                        )
                        gt.append(gsb)

                    attn = []
                    for n in range(NT):
                        pl = pool_psL.tile([128, L], F32, tag="pl")
                        b0, b1 = max(n - 1, 0), min(n + 2, NT)
                        spans = [(128 * b0, 128 * b1, 64)]
                        if 128 * (n + 2) < L:
                            spans.append((128 * (n + 2), L, 65))
                        if n - 1 > 0:
                            spans.append((0, 128 * (n - 1), 66))
                        for s0, s1, kk in spans:
                            c0 = s0
                            while c0 < s1:
                                c1 = min(s1, (c0 // 512 + 1) * 512)
                                nc.tensor.matmul(
                                    pl[:, c0:c1],
                                    k[0:kk, 128 * n : 128 * n + 128],
                                    q[0:kk, c0:c1],
                                    start=True,
                                    stop=True,
                                )
                                c0 = c1
                        at = pool_attn.tile([128, L], BF16, tag="at")
                        nc.scalar.activation(at[:], pl[:], EXP, bias=mk_sb[:, n : n + 1])
                        for m in range(b0, b1):
                            nc.vector.tensor_mul(
                                at[:, 128 * m : 128 * m + 128],
                                at[:, 128 * m : 128 * m + 128],
                                gt[m][:, 128 * (n - m + 1) : 128 * (n - m + 1) + 128],
                            )
                        attn.append(at)

                    for lh in range(2):
                        pav = pool_psV.tile([65, 512], F32, tag="pav")
                        for n in range(NT):
                            nc.tensor.matmul(
                                pav[:],
                                v_sb[n][:, 65 * h : 65 * h + 65],
                                attn[n][:, 512 * lh : 512 * lh + 512],
                                start=(n == 0),
                                stop=(n == NT - 1),
                            )
                        rec = pool_s.tile([1, 512], F32, tag="rec")
                        nc.vector.reciprocal(rec[:], pav[64:65, :])
                        pbm = pool_s.tile([64, 512], F32, tag="pbm")
                        nc.gpsimd.partition_broadcast(pbm[:], rec[:])
                        nc.vector.tensor_mul(
                            ct[h // 2][
                                64 * (h % 2) : 64 * (h % 2) + 64, 512 * lh : 512 * lh + 512
                            ],
                            pav[0:64, :],
                            pbm[:],
                        )

            # =========== output projection ===========
            with (
                tc.tile_pool(name="wout", bufs=16) as pool_wo,
                tc.tile_pool(name="oo", bufs=4) as pool_o,
                tc.tile_pool(name="ops", bufs=4, space="PSUM") as pool_ops,
            ):
                for i in range(NT):
                    w_sb = [pool_wo.tile([128, 128], F32R, tag="wo", name=f"wosb{c}") for c in range(NT)]
                    for c in range(NT):
                        nc.sync.dma_start(w_sb[c][:], wo_c[c][:, 128 * i : 128 * i + 128])
                    for lh in range(2):
                        ps = pool_ops.tile([128, 512], F32, tag="ps")
                        for c in range(NT):
                            nc.tensor.matmul(
                                ps[:],
                                w_sb[c][:],
                                ct[c][:, 512 * lh : 512 * lh + 512],
                                start=(c == 0),
                                stop=(c == NT - 1),
                            )
                        ot = pool_o.tile([128, 512], F32, tag="ot")
                        nc.vector.tensor_scalar_add(ot[:], ps[:], bo_sb[:, i : i + 1])
                        nc.sync.dma_start(
                            outt[128 * i : 128 * i + 128, 512 * lh : 512 * lh + 512], ot[:]
                        )

    nc.compile()
    return nc


def _get_nc():
    global _NC
    if _NC is None:
        _NC = _build()
    return _NC


def _prep_shared(Wq, bq, Wk, bk, Wv, bv, Wo, bo, pos_emb):
    bf = ml_dtypes.bfloat16
    wq_arr = np.ascontiguousarray(np.asarray(Wq, np.float32).T / SCALE)
    wk_arr = np.ascontiguousarray(np.asarray(Wk, np.float32).T)
    wv_arr = np.ascontiguousarray(np.asarray(Wv, np.float32).T)
    wo_arr = np.ascontiguousarray(np.asarray(Wo, np.float32).T)
    bq_c = np.ascontiguousarray((np.asarray(bq, np.float32) / SCALE).reshape(NT, 128).T)
    bk_c = np.ascontiguousarray(np.asarray(bk, np.float32).reshape(NT, 128).T)
    bv_r = np.asarray(bv, np.float32).reshape(1, D)
    bo_c = np.ascontiguousarray(np.asarray(bo, np.float32).reshape(NT, 128).T)
    ep = np.asarray(pos_emb, np.float32)
    ept_arr = np.zeros((DH, EPW), np.float32)
    ept_arr[:, :JW] = ep.T
    ep2_arr = np.stack([ep[0], ep[2 * 128] - ep[0]], axis=1)
    return {
        "wq": wq_arr, "wk": wk_arr, "wv": wv_arr, "wo": wo_arr,
        "bqc": bq_c, "bkc": bk_c, "bvr": bv_r, "boc": bo_c,
        "ept": ept_arr.astype(bf), "ep2": ep2_arr.astype(bf),
        "idn": np.eye(128, dtype=np.float32).astype(bf),
        "vob": np.ones((128, H), np.float32).astype(bf),
        "onr": np.ones((1, 128), np.float32),
    }


def kernel(x_q, x_k, x_v, mask, Wq, bq, Wk, bk, Wv, bv, Wo, bo, pos_emb):
    x_q = np.asarray(x_q, np.float32)
    x_k = np.asarray(x_k, np.float32)
    x_v = np.asarray(x_v, np.float32)
    mask = np.asarray(mask)
    nc = _get_nc()
    shared = _prep_shared(Wq, bq, Wk, bk, Wv, bv, Wo, bo, pos_emb)

    in_maps = []
    for b in range(B):
        mrow = mask[b].reshape(L).astype(bool)
        mb_c = np.ascontiguousarray(
            np.where(mrow, np.float32(-1e30), np.float32(0.0)).reshape(NT, 128).T
        )
        m = dict(shared)
        m["xqt"] = np.ascontiguousarray(x_q[b].T)
        m["xkt"] = np.ascontiguousarray(x_k[b].T)
        m["xvt"] = np.ascontiguousarray(x_v[b].T)
        m["mkb"] = mb_c
        in_maps.append(m)
    res = run_bass_kernel_spmd(nc, in_maps, core_ids=list(range(B)))
    out = np.empty((B, L, D), np.float32)
    for b in range(B):
        out[b] = res.results[b]["outt"].T
    return out
